# revision 7
# baseline (speedup 1.0000x reference)
"""CharRNN (2-layer miLSTM + big logits GEMM) Trainium2 kernel.

Sharding: data-parallel over batch across 8 cores (4 sequences each).
Each core runs the full T=128 recurrence for its 4 sequences and then
computes logits for its own 512 tokens over the FULL vocab (no
collectives). Host concatenates + row-permutes the 8 shards.

Layout is "transposed": features on partitions, batch on the free dim.
Layer 1 runs 32 steps behind layer 0; the two layers' per-step gate
math is fused into single double-width instructions using skewed
access patterns over combined (layer, ...) buffers.

Recurrence critical path is shortened by:
- q-trick: g = a*hh + c is computed as (hh + c/a (+ wic/a)) * a; the
  c/a and peephole terms are pre-accumulated into the gate PSUM via
  identity matmuls on the (otherwise idle) tensor engine, so only ONE
  vector op separates the matmul from the sigmoid.
- tanh(g_j) is folded into the gate sigmoid via tanh(x) = 2*sig(2x)-1
  (the x2 is folded into host-side weight scaling), so the scalar
  engine runs one 3-gate sigmoid instead of two serial activations.
- o2 uses per-layer scalar_tensor_tensor (wo is per-partition).
Logits PSUM->SBUF staging runs on the idle gpsimd engine in bf16 and
the output is DMAed as bf16 (host upcasts); softmax_w's 8MB DMA is
dripped into the early recurrence instead of blocking startup.
"""

import numpy as np
from contextlib import ExitStack

V, E, L, B, T = 32000, 128, 2, 32, 128
G = 4 * E
P = 128
NCORES = 8
BL = B // NCORES          # 4 sequences per core
NTOK = BL * T             # 512 tokens per core
FORGET_BIAS = 1.0
NB = 4                    # pipeline blocks (32 steps / 128 tokens each)
SPB = T // NB             # steps per block = 32
TPB = SPB * BL            # tokens per block = 128
NT_FULL = V // 512        # 62 full 512-wide logits n-tiles
NT_LAST = V - NT_FULL * 512
N_NT = NT_FULL + 1        # 63 n-tiles

_cache = {}


def _build(use_smax_bias):
    import concourse.bass as bass
    import concourse.tile as tile
    import concourse.mybir as mybir
    from concourse import bacc
    from concourse.bass import IndirectOffsetOnAxis
    from concourse.masks import make_identity

    dt = mybir.dt
    AF = mybir.ActivationFunctionType
    OP = mybir.AluOpType

    nc = bacc.Bacc("TRN2", target_bir_lowering=False, debug=False,
                   num_devices=NCORES)

    ids_d = nc.dram_tensor("ids", (P, BL), dt.int32, kind="ExternalInput")
    emb_d = nc.dram_tensor("emb", (V, E), dt.float32, kind="ExternalInput")
    wxa_d = nc.dram_tensor("wxa", (P, L, G), dt.bfloat16, kind="ExternalInput")
    wxc_d = nc.dram_tensor("wxc", (P, L, G), dt.bfloat16, kind="ExternalInput")
    wh_d = nc.dram_tensor("wh", (P, L, G), dt.bfloat16, kind="ExternalInput")
    b2t_d = nc.dram_tensor("b2t", (P, L, 4), dt.float32, kind="ExternalInput")
    bft_d = nc.dram_tensor("bft", (P, L, 4), dt.float32, kind="ExternalInput")
    pep_d = nc.dram_tensor("pep", (P, L, 3), dt.float32, kind="ExternalInput")
    wbif_d = nc.dram_tensor("wbif", (P, L, 2, BL), dt.float32,
                            kind="ExternalInput")
    swt_d = nc.dram_tensor("swt", (P, V), dt.bfloat16, kind="ExternalInput")
    if use_smax_bias:
        smb_d = nc.dram_tensor("smb", (1, V), dt.float32, kind="ExternalInput")
    # rows of out are in device token order (t*BL + s); host un-permutes
    out_d = nc.dram_tensor("out", (NTOK, V), dt.bfloat16,
                           kind="ExternalOutput")

    with tile.TileContext(nc) as tc, ExitStack() as ctx:
        singles = ctx.enter_context(tc.tile_pool(name="singles", bufs=1))
        big = ctx.enter_context(tc.tile_pool(name="big", bufs=1))
        stage_p = ctx.enter_context(tc.tile_pool(name="stage", bufs=6))
        rec = ctx.enter_context(tc.tile_pool(name="rec", bufs=3))
        cpool = ctx.enter_context(tc.tile_pool(name="cpool", bufs=3))
        arec_p = ctx.enter_context(tc.tile_pool(name="arec", bufs=2))
        ps_big = ctx.enter_context(
            tc.tile_pool(name="ps_big", bufs=2, space="PSUM"))
        ps_g = ctx.enter_context(
            tc.tile_pool(name="ps_g", bufs=3, space="PSUM"))
        ps_log = ctx.enter_context(
            tc.tile_pool(name="ps_log", bufs=3, space="PSUM"))

        # ---- static inputs -> SBUF (softmax_w deferred into recurrence) ----
        ids_sb = singles.tile([P, BL], dt.int32)
        nc.sync.dma_start(out=ids_sb[:, :], in_=ids_d[:, :])
        wxa_sb = singles.tile([P, L, G], dt.bfloat16)
        nc.sync.dma_start(out=wxa_sb[:, :, :], in_=wxa_d[:, :, :])
        wxc_sb = singles.tile([P, L, G], dt.bfloat16)
        nc.sync.dma_start(out=wxc_sb[:, :, :], in_=wxc_d[:, :, :])
        wh_sb = singles.tile([P, L, G], dt.bfloat16)
        nc.sync.dma_start(out=wh_sb[:, :, :], in_=wh_d[:, :, :])
        b2t_sb = singles.tile([P, L, 4], dt.float32)
        nc.sync.dma_start(out=b2t_sb[:, :, :], in_=b2t_d[:, :, :])
        bft_sb = singles.tile([P, L, 4], dt.float32)
        nc.sync.dma_start(out=bft_sb[:, :, :], in_=bft_d[:, :, :])
        pep_sb = singles.tile([P, L, 3], dt.float32)
        nc.sync.dma_start(out=pep_sb[:, :, :], in_=pep_d[:, :, :])
        wbif_sb = singles.tile([P, L, 2, BL], dt.float32)
        nc.sync.dma_start(out=wbif_sb[:, :, :, :], in_=wbif_d[:, :, :, :])
        swt_sb = singles.tile([P, V], dt.bfloat16)
        if use_smax_bias:
            smb_sb = singles.tile([1, V], dt.float32)
            nc.sync.dma_start(out=smb_sb[:, :], in_=smb_d[:, :])
            ones1 = singles.tile([1, P], dt.float32)
            nc.vector.memset(ones1[:, :], 1.0)

        ident = singles.tile([P, P], dt.float32)
        make_identity(nc, ident[:, :])

        zeros4 = singles.tile([P, BL], dt.float32)
        nc.vector.memset(zeros4[:, :], 0.0)
        zeros4h = singles.tile([P, BL], dt.bfloat16)
        nc.vector.memset(zeros4h[:, :], 0.0)

        # ---- embedding gather (tokens on partitions) + transpose ----
        x_sb = singles.tile([P, BL, E], dt.float32)
        for m in range(BL):
            nc.gpsimd.indirect_dma_start(
                out=x_sb[:, m, :], out_offset=None,
                in_=emb_d[:, :],
                in_offset=IndirectOffsetOnAxis(ap=ids_sb[:, m:m + 1], axis=0),
            )
        xT = singles.tile([P, NTOK], dt.bfloat16)
        for m in range(BL):
            pst = ps_big.tile([P, P], dt.float32, tag="psac")
            nc.tensor.transpose(pst[:, :], x_sb[:, m, :], ident[:, :])
            nc.scalar.copy(xT[:, m * P:(m + 1) * P], pst[:, :])

        # ---- combined (layer, ...) buffers ----
        a_all = big.tile([P, L, 4, NTOK], dt.float32)
        q_all = big.tile([P, L, 4, NTOK], dt.float32)   # c, then c/a in place
        wba_all = big.tile([P, L, 2, NTOK], dt.float32)  # w_{i,f}/a per token
        hT = big.tile([P, L, NTOK], dt.bfloat16)

        SKL_A = a_all.ap[1][0] - SPB * BL     # layer stride minus 32-step skew
        SKL_W = wba_all.ap[1][0] - SPB * BL
        SKL_H = hT.ap[1][0] - SPB * BL

        def a_skew(t):
            return bass.AP(a_all.tensor, a_all.offset + t * BL,
                           [a_all.ap[0], [SKL_A, 2], a_all.ap[2], [1, BL]])

        def q_skew(t):
            return bass.AP(q_all.tensor, q_all.offset + t * BL,
                           [q_all.ap[0], [SKL_A, 2], q_all.ap[2], [1, BL]])

        def wba_skew(t):
            return bass.AP(wba_all.tensor, wba_all.offset + t * BL,
                           [wba_all.ap[0], [SKL_W, 2], wba_all.ap[2],
                            [1, BL]])

        def h_skew(t):
            return bass.AP(hT.tensor, hT.offset + t * BL,
                           [hT.ap[0], [SKL_H, 2], [1, BL]])

        def c_bcast(cp):  # (P, 2, BL) pair-c -> (P, 2, 2, BL), dup gate dim
            return bass.AP(cp.tensor, cp.offset,
                           [cp.ap[0], cp.ap[1], [0, 2], cp.ap[2]])

        def c_bcast1(cp):  # (P, BL) -> (P, 2, BL), dup gate dim
            return bass.AP(cp.tensor, cp.offset,
                           [cp.ap[0], [0, 2], cp.ap[-1]])

        def wbif_bcast(l):  # (P, 2, BL) -> (P, 2, SPB, BL), dup step dim
            t = wbif_sb
            return bass.AP(t.tensor, t.offset + l * t.ap[1][0],
                           [t.ap[0], t.ap[2], [0, SPB], t.ap[3]])

        def blk4(tile_, l, j, n):  # [P, n, SPB, BL] view of block j, layer l
            return bass.AP(tile_.tensor,
                           tile_.offset + l * tile_.ap[1][0] + j * TPB,
                           [tile_.ap[0], [tile_.ap[2][0], n], [BL, SPB],
                            [1, BL]])

        def emit_ac_block(l, j):
            src = xT if l == 0 else hT[:, 0, :]
            blk = slice(j * TPB, (j + 1) * TPB)
            for k in range(4):
                psa = ps_big.tile([P, TPB], dt.float32, tag="psac")
                nc.tensor.matmul(psa[:, :], wxa_sb[:, l, k * P:(k + 1) * P],
                                 src[:, blk])
                nc.scalar.activation(a_all[:, l, k, blk], psa[:, :],
                                     AF.Identity, bias=b2t_sb[:, l, k:k + 1])
                psc = ps_big.tile([P, TPB], dt.float32, tag="psac")
                nc.tensor.matmul(psc[:, :], wxc_sb[:, l, k * P:(k + 1) * P],
                                 src[:, blk])
                nc.vector.tensor_scalar_add(q_all[:, l, k, blk], psc[:, :],
                                            bft_sb[:, l, k:k + 1])
            arec = arec_p.tile([P, 4, TPB], dt.float32, tag="arec")
            nc.vector.reciprocal(arec[:, :, :], a_all[:, l, :, blk])
            nc.vector.tensor_tensor(q_all[:, l, :, blk], q_all[:, l, :, blk],
                                    arec[:, :, :], op=OP.mult)
            nc.vector.tensor_tensor(
                blk4(wba_all, l, j, 2),
                bass.AP(arec.tensor, arec.offset,
                        [arec.ap[0], [arec.ap[1][0], 2], [BL, SPB], [1, BL]]),
                wbif_bcast(l), op=OP.mult)

        # recurrence state
        cpair_prev = None          # AP (P, 2, BL): [c0_t, c1_{t-32}]
        h_prev = [zeros4h[:, :], zeros4h[:, :]]

        def emit_step_single(l, t, zero_other=False):
            # one-layer step (pipeline head/tail); state kept in pair tiles
            nonlocal cpair_prev
            tb = slice(t * BL, (t + 1) * BL)
            cp = zeros4[:, :] if cpair_prev is None else cpair_prev[:, l, :]
            rt = rec.tile([P, 2, BL], dt.float32, tag="rt1")
            nc.vector.tensor_tensor(rt[:, :, :], wba_all[:, l, :, tb],
                                    c_bcast1(cp), op=OP.mult)
            psgp = ps_g.tile([P, 2, 4, BL], dt.float32, tag="psg")
            psg = psgp[:, 0, :, :]
            nc.tensor.matmul(psg[:, :, :], ident[:, :], q_all[:, l, :, tb],
                             start=True, stop=False, skip_group_check=True)
            nc.tensor.matmul(psg[:, 0:2, :], ident[:, :], rt[:, :, :],
                             start=False, stop=False, skip_group_check=True)
            for k in range(4):
                nc.tensor.matmul(psg[:, k, :],
                                 wh_sb[:, l, k * P:(k + 1) * P],
                                 h_prev[l], start=False, stop=(k == 3),
                                 skip_group_check=True)
            g = rec.tile([P, 4, BL], dt.float32, tag="g1")
            nc.vector.tensor_tensor(g[:, :, :], psg[:, :, :],
                                    a_all[:, l, :, tb], op=OP.mult)
            s3 = rec.tile([P, 3, BL], dt.float32, tag="s31")
            nc.scalar.activation(s3[:, :, :], g[:, 0:3, :], AF.Sigmoid)
            m = rec.tile([P, BL], dt.float32, tag="m1")
            nc.vector.tensor_tensor(m[:, :], s3[:, 0, :], s3[:, 2, :],
                                    op=OP.mult)
            v = rec.tile([P, BL], dt.float32, tag="v1")
            nc.vector.tensor_tensor(v[:, :], s3[:, 1, :], cp, op=OP.mult)
            u = rec.tile([P, BL], dt.float32, tag="u1")
            nc.vector.scalar_tensor_tensor(u[:, :], m[:, :], 2.0, s3[:, 0, :],
                                           op0=OP.mult, op1=OP.subtract)
            cn = cpool.tile([P, 2, BL], dt.float32, tag="cn")
            nc.vector.tensor_tensor(cn[:, l, :], u[:, :], v[:, :], op=OP.add)
            if zero_other:
                nc.vector.memset(cn[:, 1 - l, :], 0.0)
            o2 = rec.tile([P, BL], dt.float32, tag="o21")
            nc.vector.scalar_tensor_tensor(
                o2[:, :], cn[:, l, :], pep_sb[:, l, 2:3], g[:, 3, :],
                op0=OP.mult, op1=OP.add)
            tc_ = rec.tile([P, BL], dt.float32, tag="tc1")
            nc.scalar.activation(tc_[:, :], cn[:, l, :], AF.Tanh)
            so = rec.tile([P, BL], dt.float32, tag="so1")
            nc.scalar.activation(so[:, :], o2[:, :], AF.Sigmoid)
            nc.vector.tensor_tensor(hT[:, l, tb], so[:, :], tc_[:, :],
                                    op=OP.mult)
            cpair_prev = cn[:, :, :]
            h_prev[l] = hT[:, l, tb]

        def emit_pair(t0):
            # fused: layer0 step t0 + layer1 step t0-32
            nonlocal cpair_prev
            t1 = t0 - SPB
            cp = cpair_prev
            rt = rec.tile([P, 2, 2, BL], dt.float32, tag="rt")
            nc.vector.tensor_tensor(rt[:, :, :, :], wba_skew(t0),
                                    c_bcast(cp), op=OP.mult)
            psg = ps_g.tile([P, 2, 4, BL], dt.float32, tag="psg")
            nc.tensor.matmul(psg[:, :, :, :], ident[:, :], q_skew(t0),
                             start=True, stop=False, skip_group_check=True)
            nc.tensor.matmul(psg[:, :, 0:2, :], ident[:, :], rt[:, :, :, :],
                             start=False, stop=False, skip_group_check=True)
            for li in (0, 1):
                for k in range(4):
                    nc.tensor.matmul(
                        psg[:, li, k, :], wh_sb[:, li, k * P:(k + 1) * P],
                        h_prev[li], start=False,
                        stop=(li == 1 and k == 3), skip_group_check=True)
            g = rec.tile([P, 2, 4, BL], dt.float32, tag="gp")
            nc.vector.tensor_tensor(g[:, :, :, :], psg[:, :, :, :],
                                    a_skew(t0), op=OP.mult)
            s3 = rec.tile([P, 2, 3, BL], dt.float32, tag="s3p")
            nc.scalar.activation(s3[:, :, :, :], g[:, :, 0:3, :], AF.Sigmoid)
            m = rec.tile([P, 2, BL], dt.float32, tag="mp")
            nc.vector.tensor_tensor(m[:, :, :], s3[:, :, 0, :],
                                    s3[:, :, 2, :], op=OP.mult)
            v = rec.tile([P, 2, BL], dt.float32, tag="vp")
            nc.vector.tensor_tensor(v[:, :, :], s3[:, :, 1, :], cp,
                                    op=OP.mult)
            u = rec.tile([P, 2, BL], dt.float32, tag="up")
            nc.vector.scalar_tensor_tensor(u[:, :, :], m[:, :, :], 2.0,
                                           s3[:, :, 0, :],
                                           op0=OP.mult, op1=OP.subtract)
            cn = cpool.tile([P, 2, BL], dt.float32, tag="cn")
            nc.vector.tensor_tensor(cn[:, :, :], u[:, :, :], v[:, :, :],
                                    op=OP.add)
            o2 = rec.tile([P, 2, BL], dt.float32, tag="o2p")
            for li in (0, 1):
                nc.vector.scalar_tensor_tensor(
                    o2[:, li, :], cn[:, li, :], pep_sb[:, li, 2:3],
                    g[:, li, 3, :], op0=OP.mult, op1=OP.add)
            tc_ = rec.tile([P, 2, BL], dt.float32, tag="tcp")
            nc.scalar.activation(tc_[:, :, :], cn[:, :, :], AF.Tanh)
            so = rec.tile([P, 2, BL], dt.float32, tag="sop")
            nc.scalar.activation(so[:, :, :], o2[:, :, :], AF.Sigmoid)
            nc.vector.tensor_tensor(h_skew(t0), so[:, :, :], tc_[:, :, :],
                                    op=OP.mult)
            cpair_prev = cn[:, :, :]
            h_prev[0] = hT[:, 0, t0 * BL:(t0 + 1) * BL]
            h_prev[1] = hT[:, 1, t1 * BL:(t1 + 1) * BL]

        def emit_logits_ntile(k, n, eng):
            n0 = n * 512
            nn = 512 if n < NT_FULL else NT_LAST
            ps = ps_log.tile([P, 512], dt.float32)
            nc.tensor.matmul(ps[:, 0:nn], hT[:, 1, k * TPB:(k + 1) * TPB],
                             swt_sb[:, n0:n0 + nn],
                             start=True, stop=not use_smax_bias)
            if use_smax_bias:
                nc.tensor.matmul(ps[:, 0:nn], ones1[:, :],
                                 smb_sb[:, n0:n0 + nn], start=False, stop=True)
            st = stage_p.tile([P, 512], dt.bfloat16)
            if eng == 0:
                nc.vector.tensor_copy(st[:, 0:nn], ps[:, 0:nn])
            else:
                nc.scalar.copy(st[:, 0:nn], ps[:, 0:nn])
            nc.sync.dma_start(
                out=out_d[k * TPB:(k + 1) * TPB, n0:n0 + nn],
                in_=st[:, 0:nn])

        # layer-0 A/C for all tokens (x fully available)
        for j in range(NB):
            emit_ac_block(0, j)

        # ---- pipelined recurrence + logits ----
        pending = []
        ne = 0
        for jj in range(NB + 1):
            for i in range(SPB):
                if jj == 0:
                    if i % 4 == 0:     # drip softmax_w in 8 chunks of 4000
                        q8 = i // 4
                        nc.sync.dma_start(
                            out=swt_sb[:, q8 * 4000:(q8 + 1) * 4000],
                            in_=swt_d[:, q8 * 4000:(q8 + 1) * 4000])
                    emit_step_single(0, i, zero_other=(i == SPB - 1))
                elif jj < NB:
                    emit_pair(jj * SPB + i)
                else:
                    emit_step_single(1, (NB - 1) * SPB + i)
                for _ in range(2):
                    if ne < len(pending):
                        k, n = pending[ne]
                        emit_logits_ntile(k, n, ne % 2)
                        ne += 1
            if jj < NB:
                emit_ac_block(1, jj)
            if jj >= 1:
                pending.extend(((jj - 1, n) for n in range(N_NT)))
        while ne < len(pending):
            k, n = pending[ne]
            emit_logits_ntile(k, n, ne % 2)
            ne += 1

    nc.compile()
    return nc


def _prep_inputs(input_data, embedding, Wx, Wh, alpha, beta1, beta2, bias,
                 wi, wf, wo, softmax_w, softmax_b):
    import ml_dtypes
    bf16 = ml_dtypes.bfloat16
    f32 = np.float32
    input_data = np.asarray(input_data, np.int32)
    embedding = np.ascontiguousarray(np.asarray(embedding, f32))
    Wx = np.asarray(Wx, f32)
    Wh = np.asarray(Wh, f32)
    alpha = np.asarray(alpha, f32)
    beta1 = np.asarray(beta1, f32)
    beta2 = np.asarray(beta2, f32)
    bias = np.asarray(bias, f32)
    wi = np.asarray(wi, f32)
    wf = np.asarray(wf, f32)
    wo = np.asarray(wo, f32)
    softmax_w = np.asarray(softmax_w, f32)
    softmax_b = np.asarray(softmax_b, f32)

    gperm = [0, 2, 1, 3]   # reference order i,j,f,o -> device order i,f,j,o

    def permG(a):
        r = a.reshape(*a.shape[:-1], 4, E)
        return np.ascontiguousarray(r[..., gperm, :].reshape(*a.shape))

    WxA = permG(Wx * alpha[:, None, :])
    WxC = permG(Wx * beta1[:, None, :])
    Whp = permG(Wh)
    b2p = permG(beta2)
    bp = permG(bias).copy()
    bp[:, E:2 * E] += FORGET_BIAS          # f-chunk in [i|f|j|o] order
    # fold tanh(x) = 2*sigmoid(2x)-1 for gate j: scale both the a-side
    # (alpha*xh + beta2) and the c-side (beta1*xh + bias) of chunk j by 2
    WxA[:, :, 2 * E:3 * E] *= 2.0
    b2p[:, 2 * E:3 * E] *= 2.0
    WxC[:, :, 2 * E:3 * E] *= 2.0
    bp[:, 2 * E:3 * E] *= 2.0

    def to_elg(a):
        return np.ascontiguousarray(np.transpose(a, (1, 0, 2)))

    def to_plk(a):
        return np.ascontiguousarray(
            np.transpose(a.reshape(L, 4, E), (2, 0, 1)))

    pep = np.ascontiguousarray(
        np.transpose(np.stack([wi, wf, wo], axis=1), (2, 0, 1)))  # (E, L, 3)
    wbif = np.ascontiguousarray(np.broadcast_to(
        np.transpose(np.stack([wi, wf], axis=1), (2, 0, 1))[:, :, :, None],
        (E, L, 2, BL))).astype(f32)

    swt = np.ascontiguousarray(softmax_w.T)
    use_smax_bias = bool(np.any(softmax_b))

    common = {
        "emb": embedding,
        "wxa": to_elg(WxA).astype(bf16), "wxc": to_elg(WxC).astype(bf16),
        "wh": to_elg(Whp).astype(bf16),
        "b2t": to_plk(b2p), "bft": to_plk(bp), "pep": pep,
        "wbif": wbif,
        "swt": swt.astype(bf16),
    }
    if use_smax_bias:
        common["smb"] = softmax_b.reshape(1, V)

    tok = np.arange(NTOK)
    tt_, ss_ = tok // BL, tok % BL
    in_maps = []
    for c in range(NCORES):
        flat = input_data[BL * c + ss_, tt_]
        ids_pm = np.ascontiguousarray(flat.reshape(BL, P).T.astype(np.int32))
        in_maps.append({"ids": ids_pm, **common})
    return in_maps, use_smax_bias


def _run(in_maps, use_smax_bias, trace=False, tmpdir=None):
    from concourse.bass_utils import run_bass_kernel_spmd
    key = use_smax_bias
    if key not in _cache:
        _cache[key] = _build(use_smax_bias)
    nc = _cache[key]
    return run_bass_kernel_spmd(nc, in_maps, core_ids=list(range(NCORES)),
                                trace=trace, tmpdir=tmpdir)


def kernel(**inputs):
    in_maps, use_smax_bias = _prep_inputs(**inputs)
    res = _run(in_maps, use_smax_bias, trace=False)
    # device rows are token order (t*BL + s); reference rows are s*T + t
    tok = np.arange(NTOK)
    row = (tok % BL) * T + tok // BL
    out = np.empty((B * T, V), np.float32)
    for c in range(NCORES):
        out[c * NTOK + row] = np.asarray(res.results[c]["out"],
                                         dtype=np.float32)
    return out


# revision 8
# speedup vs baseline: 1.0372x; 1.0372x over previous
"""CharRNN (2-layer miLSTM + big logits GEMM) Trainium2 kernel.

Sharding: data-parallel over batch across 8 cores (4 sequences each).
Each core runs the full T=128 recurrence for its 4 sequences and then
computes logits for its own 512 tokens over the FULL vocab (no
collectives). Host concatenates + row-permutes the 8 shards.

Layout is "transposed": features on partitions, batch on the free dim.
Layer 1 runs 32 steps behind layer 0; the two layers' per-step gate
math is fused into single double-width instructions using skewed
access patterns over combined (layer, ...) buffers.

Recurrence critical path optimizations:
- q-trick: g = a*hh + c is computed as (hh + c/a (+ wic/a)) * a; the
  c/a and peephole terms are pre-accumulated into the gate PSUM via
  identity matmuls on the (otherwise idle) tensor engine, so only ONE
  vector op separates the matmul from the sigmoid.
- tanh(g_j) is folded into the gate sigmoid via tanh(x) = 2*sig(2x)-1
  (the x2 is folded into host-side weight scaling), so the scalar
  engine runs one 3-gate sigmoid instead of two serial activations.
- o2 uses per-layer scalar_tensor_tensor (wo is per-partition).
- the peephole r_t = (w_if/a)*c_{t-1} vector op for step t+1 is issued
  during step t (right after c_new) so the identity-accumulate matmul
  never waits on the vector queue.
- layer-1 a/c/q preprocessing is emitted in 16-step parts right after
  the producing h's, ~2 blocks before consumption (no boundary stall),
  and 1/a uses the fast approximate reciprocal.
Logits staging copies alternate vector/scalar, are cast to bf16 and
DMAed as bf16 (host upcasts); softmax_w's 8MB DMA is dripped into the
early recurrence instead of blocking startup.
"""

import numpy as np
from contextlib import ExitStack

V, E, L, B, T = 32000, 128, 2, 32, 128
G = 4 * E
P = 128
NCORES = 8
BL = B // NCORES          # 4 sequences per core
NTOK = BL * T             # 512 tokens per core
FORGET_BIAS = 1.0
NB = 4                    # pipeline blocks (32 steps / 128 tokens each)
SPB = T // NB             # steps per block = 32
TPB = SPB * BL            # tokens per block = 128
PRT = 16                  # layer-1 a/c part size in steps
NT_FULL = V // 512        # 62 full 512-wide logits n-tiles
NT_LAST = V - NT_FULL * 512
N_NT = NT_FULL + 1        # 63 n-tiles

_cache = {}


def _build(use_smax_bias):
    import concourse.bass as bass
    import concourse.tile as tile
    import concourse.mybir as mybir
    from concourse import bacc
    from concourse.bass import IndirectOffsetOnAxis
    from concourse.masks import make_identity

    dt = mybir.dt
    AF = mybir.ActivationFunctionType
    OP = mybir.AluOpType

    nc = bacc.Bacc("TRN2", target_bir_lowering=False, debug=False,
                   num_devices=NCORES)

    ids_d = nc.dram_tensor("ids", (P, BL), dt.int32, kind="ExternalInput")
    emb_d = nc.dram_tensor("emb", (V, E), dt.float32, kind="ExternalInput")
    wxa_d = nc.dram_tensor("wxa", (P, L, G), dt.bfloat16, kind="ExternalInput")
    wxc_d = nc.dram_tensor("wxc", (P, L, G), dt.bfloat16, kind="ExternalInput")
    wh_d = nc.dram_tensor("wh", (P, L, G), dt.bfloat16, kind="ExternalInput")
    b2t_d = nc.dram_tensor("b2t", (P, L, 4), dt.float32, kind="ExternalInput")
    bft_d = nc.dram_tensor("bft", (P, L, 4), dt.float32, kind="ExternalInput")
    pep_d = nc.dram_tensor("pep", (P, L, 3), dt.float32, kind="ExternalInput")
    wbif_d = nc.dram_tensor("wbif", (P, L, 2, BL), dt.float32,
                            kind="ExternalInput")
    swt_d = nc.dram_tensor("swt", (P, V), dt.bfloat16, kind="ExternalInput")
    if use_smax_bias:
        smb_d = nc.dram_tensor("smb", (1, V), dt.float32, kind="ExternalInput")
    # rows of out are in device token order (t*BL + s); host un-permutes
    out_d = nc.dram_tensor("out", (NTOK, V), dt.bfloat16,
                           kind="ExternalOutput")

    with tile.TileContext(nc) as tc, ExitStack() as ctx:
        singles = ctx.enter_context(tc.tile_pool(name="singles", bufs=1))
        big = ctx.enter_context(tc.tile_pool(name="big", bufs=1))
        stage_p = ctx.enter_context(tc.tile_pool(name="stage", bufs=6))
        rec = ctx.enter_context(tc.tile_pool(name="rec", bufs=3))
        cpool = ctx.enter_context(tc.tile_pool(name="cpool", bufs=3))
        arec_p = ctx.enter_context(tc.tile_pool(name="arec", bufs=2))
        ps_big = ctx.enter_context(
            tc.tile_pool(name="ps_big", bufs=2, space="PSUM"))
        ps_g = ctx.enter_context(
            tc.tile_pool(name="ps_g", bufs=3, space="PSUM"))
        ps_log = ctx.enter_context(
            tc.tile_pool(name="ps_log", bufs=3, space="PSUM"))

        # ---- static inputs -> SBUF (softmax_w deferred into recurrence) ----
        ids_sb = singles.tile([P, BL], dt.int32)
        nc.sync.dma_start(out=ids_sb[:, :], in_=ids_d[:, :])
        wxa_sb = singles.tile([P, L, G], dt.bfloat16)
        nc.sync.dma_start(out=wxa_sb[:, :, :], in_=wxa_d[:, :, :])
        wxc_sb = singles.tile([P, L, G], dt.bfloat16)
        nc.sync.dma_start(out=wxc_sb[:, :, :], in_=wxc_d[:, :, :])
        wh_sb = singles.tile([P, L, G], dt.bfloat16)
        nc.sync.dma_start(out=wh_sb[:, :, :], in_=wh_d[:, :, :])
        b2t_sb = singles.tile([P, L, 4], dt.float32)
        nc.sync.dma_start(out=b2t_sb[:, :, :], in_=b2t_d[:, :, :])
        bft_sb = singles.tile([P, L, 4], dt.float32)
        nc.sync.dma_start(out=bft_sb[:, :, :], in_=bft_d[:, :, :])
        pep_sb = singles.tile([P, L, 3], dt.float32)
        nc.sync.dma_start(out=pep_sb[:, :, :], in_=pep_d[:, :, :])
        wbif_sb = singles.tile([P, L, 2, BL], dt.float32)
        nc.sync.dma_start(out=wbif_sb[:, :, :, :], in_=wbif_d[:, :, :, :])
        swt_sb = singles.tile([P, V], dt.bfloat16)
        if use_smax_bias:
            smb_sb = singles.tile([1, V], dt.float32)
            nc.sync.dma_start(out=smb_sb[:, :], in_=smb_d[:, :])
            ones1 = singles.tile([1, P], dt.float32)
            nc.vector.memset(ones1[:, :], 1.0)

        ident = singles.tile([P, P], dt.float32)
        make_identity(nc, ident[:, :])

        zeros4 = singles.tile([P, BL], dt.float32)
        nc.vector.memset(zeros4[:, :], 0.0)
        zeros4h = singles.tile([P, BL], dt.bfloat16)
        nc.vector.memset(zeros4h[:, :], 0.0)

        # ---- embedding gather (tokens on partitions) + transpose ----
        x_sb = singles.tile([P, BL, E], dt.float32)
        for m in range(BL):
            nc.gpsimd.indirect_dma_start(
                out=x_sb[:, m, :], out_offset=None,
                in_=emb_d[:, :],
                in_offset=IndirectOffsetOnAxis(ap=ids_sb[:, m:m + 1], axis=0),
            )
        xT = singles.tile([P, NTOK], dt.bfloat16)
        for m in range(BL):
            pst = ps_big.tile([P, P], dt.float32, tag="psac")
            nc.tensor.transpose(pst[:, :], x_sb[:, m, :], ident[:, :])
            nc.scalar.copy(xT[:, m * P:(m + 1) * P], pst[:, :])

        # ---- combined (layer, ...) buffers ----
        a_all = big.tile([P, L, 4, NTOK], dt.float32)
        q_all = big.tile([P, L, 4, NTOK], dt.float32)   # c, then c/a in place
        wba_all = big.tile([P, L, 2, NTOK], dt.float32)  # w_{i,f}/a per token
        hT = big.tile([P, L, NTOK], dt.bfloat16)

        SKL_A = a_all.ap[1][0] - SPB * BL     # layer stride minus 32-step skew
        SKL_W = wba_all.ap[1][0] - SPB * BL
        SKL_H = hT.ap[1][0] - SPB * BL

        def a_skew(t):
            return bass.AP(a_all.tensor, a_all.offset + t * BL,
                           [a_all.ap[0], [SKL_A, 2], a_all.ap[2], [1, BL]])

        def q_skew(t):
            return bass.AP(q_all.tensor, q_all.offset + t * BL,
                           [q_all.ap[0], [SKL_A, 2], q_all.ap[2], [1, BL]])

        def wba_skew(t):
            return bass.AP(wba_all.tensor, wba_all.offset + t * BL,
                           [wba_all.ap[0], [SKL_W, 2], wba_all.ap[2],
                            [1, BL]])

        def h_skew(t):
            return bass.AP(hT.tensor, hT.offset + t * BL,
                           [hT.ap[0], [SKL_H, 2], [1, BL]])

        def c_bcast(cp):  # (P, 2, BL) pair-c -> (P, 2, 2, BL), dup gate dim
            return bass.AP(cp.tensor, cp.offset,
                           [cp.ap[0], cp.ap[1], [0, 2], cp.ap[2]])

        def c_bcast1(cp):  # (P, BL) -> (P, 2, BL), dup gate dim
            return bass.AP(cp.tensor, cp.offset,
                           [cp.ap[0], [0, 2], cp.ap[-1]])

        def emit_ac_part(l, st0, ns):
            # a/c/q/wba for layer l, steps [st0, st0+ns)
            w = ns * BL
            sl = slice(st0 * BL, st0 * BL + w)
            src = xT if l == 0 else hT[:, 0, :]
            for k in range(4):
                psa = ps_big.tile([P, TPB], dt.float32, tag="psac")
                nc.tensor.matmul(psa[:, 0:w],
                                 wxa_sb[:, l, k * P:(k + 1) * P], src[:, sl])
                nc.scalar.activation(a_all[:, l, k, sl], psa[:, 0:w],
                                     AF.Identity, bias=b2t_sb[:, l, k:k + 1])
                psc = ps_big.tile([P, TPB], dt.float32, tag="psac")
                nc.tensor.matmul(psc[:, 0:w],
                                 wxc_sb[:, l, k * P:(k + 1) * P], src[:, sl])
                nc.vector.tensor_scalar_add(q_all[:, l, k, sl], psc[:, 0:w],
                                            bft_sb[:, l, k:k + 1])
            arec = arec_p.tile([P, 4, TPB], dt.float32, tag="arec")
            nc.vector.reciprocal_approx_fast(arec[:, :, 0:w],
                                             a_all[:, l, :, sl])
            nc.vector.tensor_tensor(q_all[:, l, :, sl], q_all[:, l, :, sl],
                                    arec[:, :, 0:w], op=OP.mult)
            wv = bass.AP(wba_all.tensor,
                         wba_all.offset + l * wba_all.ap[1][0] + st0 * BL,
                         [wba_all.ap[0], wba_all.ap[2], [BL, ns], [1, BL]])
            av = bass.AP(arec.tensor, arec.offset,
                         [arec.ap[0], [arec.ap[1][0], 2], [BL, ns], [1, BL]])
            wb = bass.AP(wbif_sb.tensor,
                         wbif_sb.offset + l * wbif_sb.ap[1][0],
                         [wbif_sb.ap[0], wbif_sb.ap[2], [0, ns],
                          wbif_sb.ap[3]])
            nc.vector.tensor_tensor(wv, av, wb, op=OP.mult)

        # recurrence state
        cpair_prev = None          # AP (P, 2, BL): [c0_t, c1_{t-32}]
        h_prev = [zeros4h[:, :], zeros4h[:, :]]
        rt_carry = None            # r_t tile for the NEXT step's psum acc

        def emit_step_single(l, t, zero_other=False):
            # one-layer step (pipeline head/tail); state kept in pair tiles
            nonlocal cpair_prev, rt_carry
            tb = slice(t * BL, (t + 1) * BL)
            cp = zeros4[:, :] if cpair_prev is None else cpair_prev[:, l, :]
            psgp = ps_g.tile([P, 2, 4, BL], dt.float32, tag="psg")
            psg = psgp[:, 0, :, :]
            nc.tensor.matmul(psg[:, :, :], ident[:, :], q_all[:, l, :, tb],
                             start=True, stop=False, skip_group_check=True)
            if rt_carry is not None:
                nc.tensor.matmul(psg[:, 0:2, :], ident[:, :], rt_carry,
                                 start=False, stop=False,
                                 skip_group_check=True)
            for k in range(4):
                nc.tensor.matmul(psg[:, k, :],
                                 wh_sb[:, l, k * P:(k + 1) * P],
                                 h_prev[l], start=False, stop=(k == 3),
                                 skip_group_check=True)
            g = rec.tile([P, 4, BL], dt.float32, tag="g1")
            nc.vector.tensor_tensor(g[:, :, :], psg[:, :, :],
                                    a_all[:, l, :, tb], op=OP.mult)
            s3 = rec.tile([P, 3, BL], dt.float32, tag="s31")
            nc.scalar.activation(s3[:, :, :], g[:, 0:3, :], AF.Sigmoid)
            m = rec.tile([P, BL], dt.float32, tag="m1")
            nc.vector.tensor_tensor(m[:, :], s3[:, 0, :], s3[:, 2, :],
                                    op=OP.mult)
            v = rec.tile([P, BL], dt.float32, tag="v1")
            nc.vector.tensor_tensor(v[:, :], s3[:, 1, :], cp, op=OP.mult)
            u = rec.tile([P, BL], dt.float32, tag="u1")
            nc.vector.scalar_tensor_tensor(u[:, :], m[:, :], 2.0, s3[:, 0, :],
                                           op0=OP.mult, op1=OP.subtract)
            cn = cpool.tile([P, 2, BL], dt.float32, tag="cn")
            nc.vector.tensor_tensor(cn[:, l, :], u[:, :], v[:, :], op=OP.add)
            if zero_other:
                nc.vector.memset(cn[:, 1 - l, :], 0.0)
            o2 = rec.tile([P, BL], dt.float32, tag="o21")
            nc.vector.scalar_tensor_tensor(
                o2[:, :], cn[:, l, :], pep_sb[:, l, 2:3], g[:, 3, :],
                op0=OP.mult, op1=OP.add)
            # r for the next step, issued now so its matmul never waits
            if l == 0 and t == SPB - 1:          # fill -> first pair (t0=SPB)
                rt = rec.tile([P, 2, 2, BL], dt.float32, tag="rt")
                nc.vector.tensor_tensor(rt[:, :, :, :], wba_skew(SPB),
                                        c_bcast(cn[:, :, :]), op=OP.mult)
                rt_carry = rt[:, :, :, :]
            elif l == 1 and t == T - 1:          # last drain step
                rt_carry = None
            else:
                tb2 = slice((t + 1) * BL, (t + 2) * BL)
                rt = rec.tile([P, 2, BL], dt.float32, tag="rt")
                nc.vector.tensor_tensor(rt[:, :, :], wba_all[:, l, :, tb2],
                                        c_bcast1(cn[:, l, :]), op=OP.mult)
                rt_carry = rt[:, :, :]
            tc_ = rec.tile([P, BL], dt.float32, tag="tc1")
            nc.scalar.activation(tc_[:, :], cn[:, l, :], AF.Tanh)
            so = rec.tile([P, BL], dt.float32, tag="so1")
            nc.scalar.activation(so[:, :], o2[:, :], AF.Sigmoid)
            nc.vector.tensor_tensor(hT[:, l, tb], so[:, :], tc_[:, :],
                                    op=OP.mult)
            cpair_prev = cn[:, :, :]
            h_prev[l] = hT[:, l, tb]

        def emit_pair(t0):
            # fused: layer0 step t0 + layer1 step t0-32
            nonlocal cpair_prev, rt_carry
            t1 = t0 - SPB
            cp = cpair_prev
            psg = ps_g.tile([P, 2, 4, BL], dt.float32, tag="psg")
            nc.tensor.matmul(psg[:, :, :, :], ident[:, :], q_skew(t0),
                             start=True, stop=False, skip_group_check=True)
            nc.tensor.matmul(psg[:, :, 0:2, :], ident[:, :], rt_carry,
                             start=False, stop=False, skip_group_check=True)
            for li in (0, 1):
                for k in range(4):
                    nc.tensor.matmul(
                        psg[:, li, k, :], wh_sb[:, li, k * P:(k + 1) * P],
                        h_prev[li], start=False,
                        stop=(li == 1 and k == 3), skip_group_check=True)
            g = rec.tile([P, 2, 4, BL], dt.float32, tag="gp")
            nc.vector.tensor_tensor(g[:, :, :, :], psg[:, :, :, :],
                                    a_skew(t0), op=OP.mult)
            s3 = rec.tile([P, 2, 3, BL], dt.float32, tag="s3p")
            nc.scalar.activation(s3[:, :, :, :], g[:, :, 0:3, :], AF.Sigmoid)
            m = rec.tile([P, 2, BL], dt.float32, tag="mp")
            nc.vector.tensor_tensor(m[:, :, :], s3[:, :, 0, :],
                                    s3[:, :, 2, :], op=OP.mult)
            v = rec.tile([P, 2, BL], dt.float32, tag="vp")
            nc.vector.tensor_tensor(v[:, :, :], s3[:, :, 1, :], cp,
                                    op=OP.mult)
            u = rec.tile([P, 2, BL], dt.float32, tag="up")
            nc.vector.scalar_tensor_tensor(u[:, :, :], m[:, :, :], 2.0,
                                           s3[:, :, 0, :],
                                           op0=OP.mult, op1=OP.subtract)
            cn = cpool.tile([P, 2, BL], dt.float32, tag="cn")
            nc.vector.tensor_tensor(cn[:, :, :], u[:, :, :], v[:, :, :],
                                    op=OP.add)
            o2 = rec.tile([P, 2, BL], dt.float32, tag="o2p")
            for li in (0, 1):
                nc.vector.scalar_tensor_tensor(
                    o2[:, li, :], cn[:, li, :], pep_sb[:, li, 2:3],
                    g[:, li, 3, :], op0=OP.mult, op1=OP.add)
            # r for the next step, issued now so its matmul never waits
            if t0 + 1 < T:
                rt = rec.tile([P, 2, 2, BL], dt.float32, tag="rt")
                nc.vector.tensor_tensor(rt[:, :, :, :], wba_skew(t0 + 1),
                                        c_bcast(cn[:, :, :]), op=OP.mult)
                rt_carry = rt[:, :, :, :]
            else:                    # last pair -> first drain single (l=1)
                td = (NB - 1) * SPB
                rt = rec.tile([P, 2, BL], dt.float32, tag="rt")
                nc.vector.tensor_tensor(
                    rt[:, :, :], wba_all[:, 1, :, td * BL:(td + 1) * BL],
                    c_bcast1(cn[:, 1, :]), op=OP.mult)
                rt_carry = rt[:, :, :]
            tc_ = rec.tile([P, 2, BL], dt.float32, tag="tcp")
            nc.scalar.activation(tc_[:, :, :], cn[:, :, :], AF.Tanh)
            so = rec.tile([P, 2, BL], dt.float32, tag="sop")
            nc.scalar.activation(so[:, :, :], o2[:, :, :], AF.Sigmoid)
            nc.vector.tensor_tensor(h_skew(t0), so[:, :, :], tc_[:, :, :],
                                    op=OP.mult)
            cpair_prev = cn[:, :, :]
            h_prev[0] = hT[:, 0, t0 * BL:(t0 + 1) * BL]
            h_prev[1] = hT[:, 1, t1 * BL:(t1 + 1) * BL]

        def emit_logits_ntile(k, n, eng):
            n0 = n * 512
            nn = 512 if n < NT_FULL else NT_LAST
            ps = ps_log.tile([P, 512], dt.float32)
            nc.tensor.matmul(ps[:, 0:nn], hT[:, 1, k * TPB:(k + 1) * TPB],
                             swt_sb[:, n0:n0 + nn],
                             start=True, stop=not use_smax_bias)
            if use_smax_bias:
                nc.tensor.matmul(ps[:, 0:nn], ones1[:, :],
                                 smb_sb[:, n0:n0 + nn], start=False, stop=True)
            st = stage_p.tile([P, 512], dt.bfloat16)
            if eng == 0:
                nc.vector.tensor_copy(st[:, 0:nn], ps[:, 0:nn])
            else:
                nc.scalar.copy(st[:, 0:nn], ps[:, 0:nn])
            nc.sync.dma_start(
                out=out_d[k * TPB:(k + 1) * TPB, n0:n0 + nn],
                in_=st[:, 0:nn])

        # layer-0 A/C for all tokens (x fully available)
        for j in range(NB):
            emit_ac_part(0, j * SPB, SPB)

        # ---- pipelined recurrence + logits ----
        pending = []
        ne = 0
        for jj in range(NB + 1):
            for i in range(SPB):
                t = jj * SPB + i
                if jj == 0:
                    if i % 4 == 0:     # drip softmax_w in 8 chunks of 4000
                        q8 = i // 4
                        nc.sync.dma_start(
                            out=swt_sb[:, q8 * 4000:(q8 + 1) * 4000],
                            in_=swt_d[:, q8 * 4000:(q8 + 1) * 4000])
                    emit_step_single(0, i, zero_other=(i == SPB - 1))
                elif jj < NB:
                    emit_pair(t)
                else:
                    emit_step_single(1, t - SPB)
                # layer-1 a/c parts follow the producing l0 h's by 0 steps
                # and precede their consumers by >= SPB-PRT steps
                if jj < NB and (t + 1) % PRT == 0:
                    emit_ac_part(1, t + 1 - PRT, PRT)
                for _ in range(2):
                    if ne < len(pending):
                        k, n = pending[ne]
                        emit_logits_ntile(k, n, ne % 2)
                        ne += 1
            if jj >= 1:
                pending.extend(((jj - 1, n) for n in range(N_NT)))
        while ne < len(pending):
            k, n = pending[ne]
            emit_logits_ntile(k, n, ne % 2)
            ne += 1

    nc.compile()
    return nc


def _prep_inputs(input_data, embedding, Wx, Wh, alpha, beta1, beta2, bias,
                 wi, wf, wo, softmax_w, softmax_b):
    import ml_dtypes
    bf16 = ml_dtypes.bfloat16
    f32 = np.float32
    input_data = np.asarray(input_data, np.int32)
    embedding = np.ascontiguousarray(np.asarray(embedding, f32))
    Wx = np.asarray(Wx, f32)
    Wh = np.asarray(Wh, f32)
    alpha = np.asarray(alpha, f32)
    beta1 = np.asarray(beta1, f32)
    beta2 = np.asarray(beta2, f32)
    bias = np.asarray(bias, f32)
    wi = np.asarray(wi, f32)
    wf = np.asarray(wf, f32)
    wo = np.asarray(wo, f32)
    softmax_w = np.asarray(softmax_w, f32)
    softmax_b = np.asarray(softmax_b, f32)

    gperm = [0, 2, 1, 3]   # reference order i,j,f,o -> device order i,f,j,o

    def permG(a):
        r = a.reshape(*a.shape[:-1], 4, E)
        return np.ascontiguousarray(r[..., gperm, :].reshape(*a.shape))

    WxA = permG(Wx * alpha[:, None, :])
    WxC = permG(Wx * beta1[:, None, :])
    Whp = permG(Wh)
    b2p = permG(beta2)
    bp = permG(bias).copy()
    bp[:, E:2 * E] += FORGET_BIAS          # f-chunk in [i|f|j|o] order
    # fold tanh(x) = 2*sigmoid(2x)-1 for gate j: scale both the a-side
    # (alpha*xh + beta2) and the c-side (beta1*xh + bias) of chunk j by 2
    WxA[:, :, 2 * E:3 * E] *= 2.0
    b2p[:, 2 * E:3 * E] *= 2.0
    WxC[:, :, 2 * E:3 * E] *= 2.0
    bp[:, 2 * E:3 * E] *= 2.0

    def to_elg(a):
        return np.ascontiguousarray(np.transpose(a, (1, 0, 2)))

    def to_plk(a):
        return np.ascontiguousarray(
            np.transpose(a.reshape(L, 4, E), (2, 0, 1)))

    pep = np.ascontiguousarray(
        np.transpose(np.stack([wi, wf, wo], axis=1), (2, 0, 1)))  # (E, L, 3)
    wbif = np.ascontiguousarray(np.broadcast_to(
        np.transpose(np.stack([wi, wf], axis=1), (2, 0, 1))[:, :, :, None],
        (E, L, 2, BL))).astype(f32)

    swt = np.ascontiguousarray(softmax_w.T)
    use_smax_bias = bool(np.any(softmax_b))

    common = {
        "emb": embedding,
        "wxa": to_elg(WxA).astype(bf16), "wxc": to_elg(WxC).astype(bf16),
        "wh": to_elg(Whp).astype(bf16),
        "b2t": to_plk(b2p), "bft": to_plk(bp), "pep": pep,
        "wbif": wbif,
        "swt": swt.astype(bf16),
    }
    if use_smax_bias:
        common["smb"] = softmax_b.reshape(1, V)

    tok = np.arange(NTOK)
    tt_, ss_ = tok // BL, tok % BL
    in_maps = []
    for c in range(NCORES):
        flat = input_data[BL * c + ss_, tt_]
        ids_pm = np.ascontiguousarray(flat.reshape(BL, P).T.astype(np.int32))
        in_maps.append({"ids": ids_pm, **common})
    return in_maps, use_smax_bias


def _run(in_maps, use_smax_bias, trace=False, tmpdir=None):
    from concourse.bass_utils import run_bass_kernel_spmd
    key = use_smax_bias
    if key not in _cache:
        _cache[key] = _build(use_smax_bias)
    nc = _cache[key]
    return run_bass_kernel_spmd(nc, in_maps, core_ids=list(range(NCORES)),
                                trace=trace, tmpdir=tmpdir)


def kernel(**inputs):
    in_maps, use_smax_bias = _prep_inputs(**inputs)
    res = _run(in_maps, use_smax_bias, trace=False)
    # device rows are token order (t*BL + s); reference rows are s*T + t
    tok = np.arange(NTOK)
    row = (tok % BL) * T + tok // BL
    out = np.empty((B * T, V), np.float32)
    for c in range(NCORES):
        out[c * NTOK + row] = np.asarray(res.results[c]["out"],
                                         dtype=np.float32)
    return out
